# revision 18
# baseline (speedup 1.0000x reference)
"""Multi-head causal attention (B=4,S=2048,D=768,H=12,HD=64) on 8 Trainium2 cores.

Sharding: 4-way head tensor-parallel (3 heads/core) x 2-way batch data-parallel
(2 batches/core).  Core c: batch group bg=c//4 (batches 2bg,2bg+1), head group
hg=c%4 (heads 3hg..3hg+2).

Per-core device program (SPMD; per-core differences come only from data):
  1. q/k projections emitted transposed (qT,kT: [64 head-dim partitions, rows]);
     v projection row-major with 64 appended ones columns per head (softmax
     denominator rides along the AV matmul replicated on psum rows 64:128).
  2. Causal attention computed transposed: S_T[k,q] = kT.T @ qT, so P=exp(S_T)
     feeds AV directly with no P transpose.  Softmax skips the running max
     (scores are O(1) at this problem's scale; exp is mathematically identical
     to the reference since softmax is shift-invariant).  The two batches of a
     head run concurrently on the PE via 64-row tile packing (row-tiled
     matmuls).  AV accumulates ctxU_T[128, q512] = sum_k vE.T @ P_T (rows
     64:128 = denominator l replicated).  Normalize: 1/l via fast-approx DVE
     reciprocal, one fused DVE multiply -- no partition broadcast, no
     single-partition ops.
  3. Per 512-row q-block (x2 batches = 1024-row chunk): ONE 8-core AllToAll
     (bf16, 192-row shards = all 3 head-pairs) at block end redistributes ctx
     so each core holds all 768 context features for its own 2x128 output
     rows; 4 senders x 192 rows = 768 = 6x128, so the received buffer carves
     into K=128 out-projection chunks that line up with the NATURAL Wp row
     order (no permutation); bias folds into the psum->sbuf copy as a DVE
     tensor-add.

The whole attention sweep is software-pipelined one k-pair deep: the score
matmuls of unit t+1 are emitted before the exp/AV of unit t, so the PE chews
on S_T[t+1] while the scalar engine runs EXP[t].  qk/v projection chunks and
out-projection blocks are interleaved between units as PE filler, scheduled
so no DMA or matmul is ever queue-gated on an unfinished collective
(collectives serialize in emission order and gate everything emitted after
them on the same queues).  Only q-block 3's own out-projection trails the
final AllToAll, split per 384-wide half so stores overlap the matmuls.

Host side only slices/casts/transposes inputs and concatenates output shards.
"""

import sys

if "/opt/trn_rl_repo" not in sys.path:
    sys.path.insert(0, "/opt/trn_rl_repo")

import numpy as np
import ml_dtypes

BF16 = ml_dtypes.bfloat16
F8E4 = ml_dtypes.float8_e4m3

B, S, D = 4, 2048, 768
H, HD = 12, 64
N_CORES = 8
BL = 2          # batches per core
HL = 3          # heads per core
R = BL * S      # 4096 rows per core
KSUB = D // 128  # 6

_CACHE = {}


def _build_nc():
    import concourse.bass as bass  # noqa: F401
    import concourse.tile as tile
    from concourse import bacc, mybir

    f32 = mybir.dt.float32
    bf16 = mybir.dt.bfloat16
    f8 = mybir.dt.float8e4
    DR = mybir.MatmulPerfMode.DoubleRow
    EXP = mybir.ActivationFunctionType.Exp

    nc = bacc.Bacc("TRN2", target_bir_lowering=False, debug=False,
                   num_devices=N_CORES)

    xT_d = nc.dram_tensor("xT", [D, R], bf16, kind="ExternalInput").ap()
    wqk_d = nc.dram_tensor("wqk", [D, 2 * HL * HD], bf16, kind="ExternalInput").ap()
    wv_d = nc.dram_tensor("wv", [D, HL * HD], bf16, kind="ExternalInput").ap()
    wp_d = nc.dram_tensor("wp", [D, D], bf16, kind="ExternalInput").ap()
    bp_d = nc.dram_tensor("bp", [1, D], bf16, kind="ExternalInput").ap()
    mk_d = nc.dram_tensor("mk", [2, 128, 1024], bf16, kind="ExternalInput").ap()
    out_d = nc.dram_tensor("out", [4, 2, 128, D], f32, kind="ExternalOutput").ap()

    RG = [[0, 1, 2, 3, 4, 5, 6, 7]]

    with tile.TileContext(nc) as tc:
        with tc.tile_pool(name="persist", bufs=1) as per, \
             tc.tile_pool(name="dram", bufs=1, space="DRAM") as dram, \
             tc.tile_pool(name="mix_ps", bufs=2, space="PSUM") as mix_ps, \
             tc.tile_pool(name="st_ps", bufs=2, space="PSUM") as st_ps, \
             tc.tile_pool(name="av_ps", bufs=2, space="PSUM") as av_ps, \
             tc.tile_pool(name="pt", bufs=8) as ptp, \
             tc.tile_pool(name="sm", bufs=4) as sm, \
             tc.tile_pool(name="agp", bufs=4) as agp, \
             tc.tile_pool(name="outp", bufs=4) as outp:
            # ---- persistent SBUF tensors -------------------------------
            wqk = per.tile([128, KSUB, 2 * HL * HD], bf16, tag="wqk")
            wv = per.tile([128, KSUB, HL * HD], bf16, tag="wv")
            xT = per.tile([128, KSUB, R], bf16, tag="xT")
            xTr = xT_d.rearrange("(o p) r -> p o r", p=128)
            for j in range(KSUB):  # consumption order for fast PE start
                nc.sync.dma_start(
                    wqk[:, j], wqk_d.rearrange("(o p) c -> p o c", p=128)[:, j])
                nc.sync.dma_start(xT[:, j, 0:256], xTr[:, j, 0:256])
            for j in range(KSUB):
                nc.sync.dma_start(xT[:, j, 256:512], xTr[:, j, 256:512])
                nc.sync.dma_start(xT[:, j, S:S + 512], xTr[:, j, S:S + 512])
                nc.sync.dma_start(
                    wv[:, j], wv_d.rearrange("(o p) c -> p o c", p=128)[:, j])
            # remaining x chunks, alternating batches (b0rc, b1rc)
            for rc in range(1, 4):
                for b in range(2):
                    r0 = b * S + rc * 512
                    for j in range(KSUB):
                        nc.sync.dma_start(xT[:, j, r0:r0 + 512],
                                          xTr[:, j, r0:r0 + 512])
            masks = per.tile([128, 2, 1024], bf16, tag="mk")
            nc.sync.dma_start(masks[:], mk_d.rearrange("o p c -> p o c"))
            wp = per.tile([128, KSUB, D], bf16, tag="wp")
            nc.sync.dma_start(wp[:], wp_d.rearrange("(o p) c -> p o c", p=128))
            bp_sb = per.tile([1, D], bf16, tag="bp")
            nc.sync.dma_start(bp_sb[:], bp_d[:])
            biasB = per.tile([128, D], bf16, tag="biasB")
            nc.gpsimd.partition_broadcast(biasB[:], bp_sb[:])

            # pair p = head p; partitions 0:64 = batch 0, 64:128 = batch 1
            qT = per.tile([128, HL, S], bf16, tag="qT")
            kT = per.tile([128, HL, S], bf16, tag="kT")
            # vE per (row-tile, pair): [v 64 cols | ones 64 cols] so the AV
            # matmul (M=128) lands the denominator replicated on psum
            # partitions 64:128
            vE = per.tile([128, 2 * 16, HL, 128], bf16, tag="vE")
            nc.vector.memset(vE[:], 1.0)

            ab_in = [dram.tile([8 * 192, 128], bf16, name=f"abi{qb}",
                               tag=f"abi{qb}") for qb in range(4)]
            ab_out = [dram.tile([8 * 192, 128], bf16, name=f"abo{qb}",
                                tag=f"abo{qb}") for qb in range(4)]

            # ---- emission helpers --------------------------------------
            def emit_qk_ct(b, rc, ct, first=False):
                r0 = b * S + rc * 512
                if True:
                    ps = mix_ps.tile([128, 512], f32, tag="mix", name="ps")
                    for j in range(KSUB):
                        nc.tensor.matmul(
                            ps[:, 0:256] if first else ps[:],
                            lhsT=wqk[:, j, ct * 128:(ct + 1) * 128],
                            rhs=xT[:, j, r0:r0 + 256] if first
                            else xT[:, j, r0:r0 + 512],
                            start=(j == 0), stop=(j == KSUB - 1))
                    if first:
                        # second half separately: lets the first chain start
                        # as soon as the leading half-chunks of x land
                        for j in range(KSUB):
                            nc.tensor.matmul(
                                ps[:, 256:512],
                                lhsT=wqk[:, j, ct * 128:(ct + 1) * 128],
                                rhs=xT[:, j, r0 + 256:r0 + 512],
                                start=(j == 0), stop=(j == KSUB - 1))
                    for half in range(2):
                        gid = 2 * ct + half
                        dest = qT if gid < 3 else kT
                        pair = gid % 3
                        nc.vector.tensor_copy(
                            dest[b * 64:(b + 1) * 64, pair,
                                 rc * 512:(rc + 1) * 512],
                            ps[half * 64:(half + 1) * 64, :])

            def emit_v_tile(b, rt):
                r0 = b * S + rt * 128
                psv = mix_ps.tile([128, HL * HD], f32, tag="mix", name="psv")
                for j in range(KSUB):
                    nc.tensor.matmul(
                        psv[:], lhsT=xT[:, j, r0:r0 + 128], rhs=wv[:, j, :],
                        start=(j == 0), stop=(j == KSUB - 1))
                nc.vector.tensor_copy(
                    vE[:, b * 16 + rt, :, 0:HD],
                    psv[:, :].rearrange("p (h c) -> p h c", c=HD))

            # ---- attention units (qb, pair, kp), software-pipelined ----
            units = [(qb, pair, kp)
                     for qb in range(4)
                     for pair in range(HL)
                     for kp in range(2 * (qb + 1))]
            NU = len(units)
            uidx = {u: i for i, u in enumerate(units)}
            st_tiles = {}   # t -> [stps_u0, stps_u1]
            av_tiles = {}   # (qb, pair) -> [avs_u0, avs_u1]

            def emit_st(t):
                qb, pair, kp = units[t]
                n_kp = 2 * (qb + 1)
                q0 = qb * 512
                o = kp - (n_kp - 2)  # diag pair offset; >=0 on diagonal
                qv0 = 256 if o == 1 else 0
                stps = [st_ps.tile([128, 2, 512], f32, tag="st",
                                   name=f"st{u}") for u in range(2)]
                st_tiles[t] = stps
                for i in range(2):
                    for u in range(2):
                        kt = 2 * kp + i
                        nc.tensor.matmul(
                            stps[u][:, i, qv0:512],
                            lhsT=kT[u * 64:(u + 1) * 64, pair,
                                    kt * 128:(kt + 1) * 128],
                            rhs=qT[u * 64:(u + 1) * 64, pair,
                                   q0 + qv0:q0 + 512],
                            start=True, stop=True)

            def emit_sm_av(t):
                qb, pair, kp = units[t]
                n_kp = 2 * (qb + 1)
                o = kp - (n_kp - 2)
                qv0 = 256 if o == 1 else 0
                if kp == 0:
                    av_tiles[(qb, pair)] = [
                        av_ps.tile([128, 512], f32, tag="av", name=f"av{u}")
                        for u in range(2)]
                avs = av_tiles[(qb, pair)]
                stps = st_tiles.pop(t)
                for u in range(2):
                    pt = ptp.tile([128, 2, 512], bf16, tag="pt")
                    nc.scalar.activation(pt[:, :, qv0:512],
                                         stps[u][:, :, qv0:512], EXP,
                                         scale=float(HD) ** -0.5)
                    if o >= 0:
                        mk2 = masks[:, o, :].rearrange("p (i c) -> p i c",
                                                       i=2)
                        nc.vector.tensor_mul(pt[:, :, qv0:512],
                                             pt[:, :, qv0:512],
                                             mk2[:, :, qv0:512])
                    for i in range(2):
                        kt = 2 * kp + i
                        # valid q range of this k-subtile (av accumulate is
                        # sub-range safe; masked-out region contributes 0)
                        qa = qv0 if o < 0 else 128 * (2 * o + i)
                        nc.tensor.matmul(
                            avs[u][:, qa:512],
                            lhsT=vE[:, u * 16 + kt, pair, :],
                            rhs=pt[:, i, qa:512],
                            start=(kp == 0 and i == 0),
                            stop=(kp == n_kp - 1 and i == 1))

            def emit_norm(qb, pair):
                avs = av_tiles.pop((qb, pair))
                a2v = ab_in[qb].rearrange("(j f) c -> f j c", f=192)
                a2v = a2v[64 * pair:64 * (pair + 1), :, :]
                for u in range(2):
                    # NB: reciprocal_approx_fast is a custom-DVE op and can
                    # NOT read PSUM -- copy the denominator to SBUF first
                    lsb = sm.tile([64, 512], f32, tag="lsb", name=f"lsb{u}")
                    nc.vector.tensor_copy(lsb[:], avs[u][64:128, :])
                    rec = sm.tile([64, 512], f32, tag="rec", name=f"rec{u}")
                    nc.vector.reciprocal_approx_fast(rec[:], lsb[:])
                    ctxn = sm.tile([64, 512], bf16, tag="ctxn",
                                   name=f"ctxn{u}")
                    nc.vector.tensor_mul(ctxn[:], avs[u][0:64, :], rec[:])
                    nc.sync.dma_start(
                        a2v[:, 4 * u:4 * u + 4, :],
                        ctxn.rearrange("p (q c) -> p q c", q=4))

            def emit_coll(ins, outs):
                nc.gpsimd.collective_compute(
                    "AllToAll", mybir.AluOpType.bypass,
                    ins=[ins[:]], outs=[outs[:]], replica_groups=RG)

            def fetch_ag(ag, qb):
                nc.sync.dma_start(
                    ag[:], ab_out[qb].rearrange("(o p) r -> p o r", p=128))

            def emit_outproj_blk(qb, blk, ag, split_dma=True):
                osb = outp.tile([128, D], f32, tag="osb")
                for nh in range(2):
                    po = mix_ps.tile([128, 384], f32, tag="mix", name="po")
                    n0 = nh * 384
                    for j in range(KSUB):
                        nc.tensor.matmul(po[:],
                                         lhsT=ag[:, 6 * blk + j, :],
                                         rhs=wp[:, j, n0:n0 + 384],
                                         start=(j == 0),
                                         stop=(j == KSUB - 1))
                    nc.vector.tensor_add(osb[:, n0:n0 + 384], po[:],
                                         biasB[:, n0:n0 + 384])
                    if split_dma:
                        nc.sync.dma_start(out_d[qb, blk, :, n0:n0 + 384],
                                          osb[:, n0:n0 + 384])
                if not split_dma:
                    nc.sync.dma_start(out_d[qb, blk], osb[:])

            # ---- software-pipelined emission ---------------------------
            # (no warmup collective: AB(0) absorbs the ncfw first-call cost;
            # its results are not needed until q-block 3's filler window)
            # prologue: everything attention qb0 needs
            for b in range(2):
                for ct in range(3):
                    emit_qk_ct(b, 0, ct, first=(b == 0 and ct == 0))
            for rt in range(4):
                emit_v_tile(0, rt)
                emit_v_tile(1, rt)

            # filler queue: (deadline=emission step, emit_fn); before each
            # unit's S_T all fillers due by then are drained (hard ordering
            # requirement: a filler must be emitted before the attention
            # that consumes its output), plus one opportunistically per
            # step to spread PE filler.
            import heapq
            fqh = []
            fqseq = [0]

            def fq_push(dl, fn):
                heapq.heappush(fqh, (dl, fqseq[0], fn))
                fqseq[0] += 1

            for rc in range(1, 4):
                for b in range(2):
                    for ct in range(3):
                        # qT/kT rows rc needed from the first unit of qb=rc;
                        # its S_T is emitted one step early
                        fq_push(uidx[(rc, 0, 0)] - 1,
                                lambda b=b, rc=rc, ct=ct:
                                emit_qk_ct(b, rc, ct))
                for rt in range(4 * rc, 4 * rc + 4):
                    for b in range(2):
                        # vE row-tile rt consumed at AV of kp=rt//2 of the
                        # first pair of qb=rc
                        fq_push(uidx[(rc, 0, max(0, rt // 2 - 1))],
                                lambda b=b, rt=rt: emit_v_tile(b, rt))

            def drain(n, due=None):
                k = 0
                while fqh and (k < n or (due is not None and fqh[0][0] <= due)):
                    heapq.heappop(fqh)[2]()
                    k += 1

            # out-projection of q-block qb: ag fetch ~half a q-block after
            # AB(qb) completes (its DMA is queue-gated on AB(qb) anyway);
            # the matmul blocks run as PE filler another half-block later
            FETCH_AT = {0: (2, 1, 1), 1: (2, 2, 2), 2: (3, 1, 0)}
            OP_AT = {0: [(3, 0, 0), (3, 0, 2)],
                     1: [(3, 0, 4), (3, 0, 6)],
                     2: [(3, 1, 3), (3, 1, 6)]}
            ags = {}

            emit_st(0)
            for t in range(NU):
                if t + 1 < NU:
                    drain(1, due=t + 1)
                    emit_st(t + 1)
                emit_sm_av(t)
                qb, pair, kp = units[t]
                if kp != 2 * (qb + 1) - 1:
                    continue
                # ---- last k-pair of (qb, pair) ----
                emit_norm(qb, pair)
                if pair == 2 and qb == 3:
                    drain(99, due=NU)   # everything out before the last coll
                if pair == 2:
                    emit_coll(ab_in[qb], ab_out[qb])
                    if qb < 3:
                        ag = agp.tile([128, 2 * KSUB, 128], bf16, tag="ag",
                                      name=f"ag{qb}")
                        ags[qb] = ag
                        fq_push(uidx[FETCH_AT[qb]],
                                lambda ag=ag, qb=qb: fetch_ag(ag, qb))
                        for blk, su in enumerate(OP_AT[qb]):
                            fq_push(uidx[su],
                                    lambda qb=qb, blk=blk, ag=ag:
                                    emit_outproj_blk(qb, blk, ag))
            # ---- epilogue: q-block 3 out-projection ---------------------
            ag3 = agp.tile([128, 2 * KSUB, 128], bf16, tag="ag", name="ag3")
            abo3 = ab_out[3].rearrange("(o p) r -> p o r", p=128)
            nc.sync.dma_start(ag3[:, 0:KSUB, :], abo3[:, 0:KSUB, :])
            nc.sync.dma_start(ag3[:, KSUB:2 * KSUB, :],
                              abo3[:, KSUB:2 * KSUB, :])
            emit_outproj_blk(3, 0, ag3, split_dma=True)
            emit_outproj_blk(3, 1, ag3, split_dma=True)

    nc.compile()
    return nc


def _get_nc():
    if "nc" not in _CACHE:
        _CACHE["nc"] = _build_nc()
    return _CACHE["nc"]


def _masks_np():
    k = np.arange(128)[:, None]
    q = np.arange(512)[None, :]
    tiles = [(q >= k + 128 * t) for t in range(4)]
    m = np.stack([np.concatenate([tiles[2 * o], tiles[2 * o + 1]], axis=1)
                  for o in range(2)])
    return m.astype(BF16)


def _prep_in_maps(x, Wq, Wk, Wv, Wp, bp):
    x = np.asarray(x, dtype=np.float32)
    mk = _masks_np()
    # Natural Wp row order: the merged per-q-block AllToAll delivers each
    # sender's 192 rows contiguously, and 4 senders x 192 = 6 x 128-row
    # matmul chunks in plain head order.
    wp_full = np.asarray(Wp).astype(BF16)
    bp_row = np.asarray(bp, dtype=np.float32).reshape(1, D).astype(BF16)
    xT_bg = []
    for bg in range(2):
        xl = x[2 * bg:2 * bg + 2].reshape(R, D)
        xT_bg.append(np.ascontiguousarray(xl.T).astype(BF16))
    wqk_hg, wv_hg = [], []
    for hg in range(4):
        hs = slice(192 * hg, 192 * (hg + 1))
        wqk_hg.append(np.concatenate(
            [np.asarray(Wq)[:, hs], np.asarray(Wk)[:, hs]], axis=1).astype(BF16))
        wv_hg.append(np.asarray(Wv)[:, hs].astype(BF16))
    in_maps = []
    for c in range(N_CORES):
        bg, hg = c // 4, c % 4
        in_maps.append({
            "xT": xT_bg[bg],
            "wqk": wqk_hg[hg],
            "wv": wv_hg[hg],
            "wp": wp_full,
            "bp": bp_row,
            "mk": mk,
        })
    return in_maps


def kernel(x, Wq, Wk, Wv, Wp, bp):
    from concourse import bass_utils

    nc = _get_nc()
    in_maps = _prep_in_maps(x, Wq, Wk, Wv, Wp, bp)
    res = bass_utils.run_bass_kernel_spmd(nc, in_maps,
                                          core_ids=list(range(N_CORES)))
    out = np.empty((B, S, D), np.float32)
    for c in range(N_CORES):
        sh = res.results[c]["out"]  # [4 chunks, 2 blocks, 128, D]
        for qb in range(4):
            for blk in range(2):
                batch = 2 * blk + c // 4
                s0 = 512 * qb + 128 * (c % 4)
                out[batch, s0:s0 + 128] = sh[qb, blk]
    return out


# revision 19
# speedup vs baseline: 1.0061x; 1.0061x over previous
"""Multi-head causal attention (B=4,S=2048,D=768,H=12,HD=64) on 8 Trainium2 cores.

Sharding: 4-way head tensor-parallel (3 heads/core) x 2-way batch data-parallel
(2 batches/core).  Core c: batch group bg=c//4 (batches 2bg,2bg+1), head group
hg=c%4 (heads 3hg..3hg+2).

Per-core device program (SPMD; per-core differences come only from data):
  1. q/k projections emitted transposed (qT,kT: [64 head-dim partitions, rows]);
     v projection row-major with 64 appended ones columns per head (softmax
     denominator rides along the AV matmul replicated on psum rows 64:128).
  2. Causal attention computed transposed: S_T[k,q] = kT.T @ qT, so P=exp(S_T)
     feeds AV directly with no P transpose.  Softmax skips the running max
     (scores are O(1) at this problem's scale; exp is mathematically identical
     to the reference since softmax is shift-invariant).  The two batches of a
     head run concurrently on the PE via 64-row tile packing (row-tiled
     matmuls).  AV accumulates ctxU_T[128, q512] = sum_k vE.T @ P_T (rows
     64:128 = denominator l replicated).  Normalize: 1/l via fast-approx DVE
     reciprocal, one fused DVE multiply -- no partition broadcast, no
     single-partition ops.
  3. Per 512-row q-block (x2 batches = 1024-row chunk): ONE 8-core AllToAll
     (bf16, 192-row shards = all 3 head-pairs) at block end redistributes ctx
     so each core holds all 768 context features for its own 2x128 output
     rows; 4 senders x 192 rows = 768 = 6x128, so the received buffer carves
     into K=128 out-projection chunks that line up with the NATURAL Wp row
     order (no permutation); bias folds into the psum->sbuf copy as a DVE
     tensor-add.

The whole attention sweep is software-pipelined one k-pair deep: the score
matmuls of unit t+1 are emitted before the exp/AV of unit t, so the PE chews
on S_T[t+1] while the scalar engine runs EXP[t].  qk/v projection chunks and
out-projection blocks are interleaved between units as PE filler, scheduled
so no DMA or matmul is ever queue-gated on an unfinished collective
(collectives serialize in emission order and gate everything emitted after
them on the same queues).  Only q-block 3's own out-projection trails the
final AllToAll, split per 384-wide half so stores overlap the matmuls.

Host side only slices/casts/transposes inputs and concatenates output shards.
"""

import sys

if "/opt/trn_rl_repo" not in sys.path:
    sys.path.insert(0, "/opt/trn_rl_repo")

import numpy as np
import ml_dtypes

BF16 = ml_dtypes.bfloat16
F8E4 = ml_dtypes.float8_e4m3

B, S, D = 4, 2048, 768
H, HD = 12, 64
N_CORES = 8
BL = 2          # batches per core
HL = 3          # heads per core
R = BL * S      # 4096 rows per core
KSUB = D // 128  # 6

_CACHE = {}


def _build_nc():
    import concourse.bass as bass  # noqa: F401
    import concourse.tile as tile
    from concourse import bacc, mybir

    f32 = mybir.dt.float32
    bf16 = mybir.dt.bfloat16
    f8 = mybir.dt.float8e4
    DR = mybir.MatmulPerfMode.DoubleRow
    EXP = mybir.ActivationFunctionType.Exp

    nc = bacc.Bacc("TRN2", target_bir_lowering=False, debug=False,
                   num_devices=N_CORES)

    xT_d = nc.dram_tensor("xT", [D, R], bf16, kind="ExternalInput").ap()
    wqk_d = nc.dram_tensor("wqk", [D, 2 * HL * HD], bf16, kind="ExternalInput").ap()
    wv_d = nc.dram_tensor("wv", [D, HL * HD], bf16, kind="ExternalInput").ap()
    wp_d = nc.dram_tensor("wp", [D, D], bf16, kind="ExternalInput").ap()
    wpP_d = nc.dram_tensor("wpP", [D, D], bf16, kind="ExternalInput").ap()
    bp_d = nc.dram_tensor("bp", [1, D], bf16, kind="ExternalInput").ap()
    mk_d = nc.dram_tensor("mk", [2, 128, 1024], bf16, kind="ExternalInput").ap()
    out_d = nc.dram_tensor("out", [4, 2, 128, D], f32, kind="ExternalOutput").ap()

    RG = [[0, 1, 2, 3, 4, 5, 6, 7]]

    with tile.TileContext(nc) as tc:
        with tc.tile_pool(name="persist", bufs=1) as per, \
             tc.tile_pool(name="dram", bufs=1, space="DRAM") as dram, \
             tc.tile_pool(name="mix_ps", bufs=2, space="PSUM") as mix_ps, \
             tc.tile_pool(name="st_ps", bufs=2, space="PSUM") as st_ps, \
             tc.tile_pool(name="av_ps", bufs=2, space="PSUM") as av_ps, \
             tc.tile_pool(name="pt", bufs=8) as ptp, \
             tc.tile_pool(name="sm", bufs=4) as sm, \
             tc.tile_pool(name="agp", bufs=4) as agp, \
             tc.tile_pool(name="outp", bufs=4) as outp:
            # ---- persistent SBUF tensors -------------------------------
            wqk = per.tile([128, KSUB, 2 * HL * HD], bf16, tag="wqk")
            wv = per.tile([128, KSUB, HL * HD], bf16, tag="wv")
            xT = per.tile([128, KSUB, R], bf16, tag="xT")
            xTr = xT_d.rearrange("(o p) r -> p o r", p=128)
            for j in range(KSUB):  # consumption order for fast PE start
                nc.sync.dma_start(
                    wqk[:, j], wqk_d.rearrange("(o p) c -> p o c", p=128)[:, j])
                nc.sync.dma_start(xT[:, j, 0:256], xTr[:, j, 0:256])
            for j in range(KSUB):
                nc.sync.dma_start(xT[:, j, 256:512], xTr[:, j, 256:512])
                nc.sync.dma_start(xT[:, j, S:S + 512], xTr[:, j, S:S + 512])
                nc.sync.dma_start(
                    wv[:, j], wv_d.rearrange("(o p) c -> p o c", p=128)[:, j])
            # remaining x chunks, alternating batches (b0rc, b1rc)
            for rc in range(1, 4):
                for b in range(2):
                    r0 = b * S + rc * 512
                    for j in range(KSUB):
                        nc.sync.dma_start(xT[:, j, r0:r0 + 512],
                                          xTr[:, j, r0:r0 + 512])
            masks = per.tile([128, 2, 1024], bf16, tag="mk")
            nc.sync.dma_start(masks[:], mk_d.rearrange("o p c -> p o c"))
            wp = per.tile([128, KSUB, D], bf16, tag="wp")
            nc.sync.dma_start(wp[:], wp_d.rearrange("(o p) c -> p o c", p=128))
            wpP = per.tile([128, KSUB, D], bf16, tag="wpP")
            nc.sync.dma_start(wpP[:],
                              wpP_d.rearrange("(o p) c -> p o c", p=128))
            bp_sb = per.tile([1, D], bf16, tag="bp")
            nc.sync.dma_start(bp_sb[:], bp_d[:])
            biasB = per.tile([128, D], bf16, tag="biasB")
            nc.gpsimd.partition_broadcast(biasB[:], bp_sb[:])

            # pair p = head p; partitions 0:64 = batch 0, 64:128 = batch 1
            qT = per.tile([128, HL, S], bf16, tag="qT")
            kT = per.tile([128, HL, S], bf16, tag="kT")
            # vE per (row-tile, pair): [v 64 cols | ones 64 cols] so the AV
            # matmul (M=128) lands the denominator replicated on psum
            # partitions 64:128
            vE = per.tile([128, 2 * 16, HL, 128], bf16, tag="vE")
            nc.vector.memset(vE[:], 1.0)

            ab_in = [dram.tile([8 * 192, 128], bf16, name=f"abi{qb}",
                               tag=f"abi{qb}") for qb in range(3)]
            ab_out = [dram.tile([8 * 192, 128], bf16, name=f"abo{qb}",
                                tag=f"abo{qb}") for qb in range(3)]
            a3a_in = dram.tile([8 * 128, 128], bf16, tag="a3ai")
            a3a_out = dram.tile([8 * 128, 128], bf16, tag="a3ao")
            a3b_in = dram.tile([8 * 64, 128], bf16, tag="a3bi")
            a3b_out = dram.tile([8 * 64, 128], bf16, tag="a3bo")

            # ---- emission helpers --------------------------------------
            def emit_qk_ct(b, rc, ct, first=False):
                r0 = b * S + rc * 512
                if True:
                    ps = mix_ps.tile([128, 512], f32, tag="mix", name="ps")
                    for j in range(KSUB):
                        nc.tensor.matmul(
                            ps[:, 0:256] if first else ps[:],
                            lhsT=wqk[:, j, ct * 128:(ct + 1) * 128],
                            rhs=xT[:, j, r0:r0 + 256] if first
                            else xT[:, j, r0:r0 + 512],
                            start=(j == 0), stop=(j == KSUB - 1))
                    if first:
                        # second half separately: lets the first chain start
                        # as soon as the leading half-chunks of x land
                        for j in range(KSUB):
                            nc.tensor.matmul(
                                ps[:, 256:512],
                                lhsT=wqk[:, j, ct * 128:(ct + 1) * 128],
                                rhs=xT[:, j, r0 + 256:r0 + 512],
                                start=(j == 0), stop=(j == KSUB - 1))
                    for half in range(2):
                        gid = 2 * ct + half
                        dest = qT if gid < 3 else kT
                        pair = gid % 3
                        nc.vector.tensor_copy(
                            dest[b * 64:(b + 1) * 64, pair,
                                 rc * 512:(rc + 1) * 512],
                            ps[half * 64:(half + 1) * 64, :])

            def emit_v_tile(b, rt):
                r0 = b * S + rt * 128
                psv = mix_ps.tile([128, HL * HD], f32, tag="mix", name="psv")
                for j in range(KSUB):
                    nc.tensor.matmul(
                        psv[:], lhsT=xT[:, j, r0:r0 + 128], rhs=wv[:, j, :],
                        start=(j == 0), stop=(j == KSUB - 1))
                nc.vector.tensor_copy(
                    vE[:, b * 16 + rt, :, 0:HD],
                    psv[:, :].rearrange("p (h c) -> p h c", c=HD))

            # ---- attention units (qb, pair, kp), software-pipelined ----
            units = [(qb, pair, kp)
                     for qb in range(4)
                     for pair in range(HL)
                     for kp in range(2 * (qb + 1))]
            NU = len(units)
            uidx = {u: i for i, u in enumerate(units)}
            st_tiles = {}   # t -> [stps_u0, stps_u1]
            av_tiles = {}   # (qb, pair) -> [avs_u0, avs_u1]

            def emit_st(t):
                qb, pair, kp = units[t]
                n_kp = 2 * (qb + 1)
                q0 = qb * 512
                o = kp - (n_kp - 2)  # diag pair offset; >=0 on diagonal
                qv0 = 256 if o == 1 else 0
                stps = [st_ps.tile([128, 2, 512], f32, tag="st",
                                   name=f"st{u}") for u in range(2)]
                st_tiles[t] = stps
                for i in range(2):
                    for u in range(2):
                        kt = 2 * kp + i
                        nc.tensor.matmul(
                            stps[u][:, i, qv0:512],
                            lhsT=kT[u * 64:(u + 1) * 64, pair,
                                    kt * 128:(kt + 1) * 128],
                            rhs=qT[u * 64:(u + 1) * 64, pair,
                                   q0 + qv0:q0 + 512],
                            start=True, stop=True)

            def emit_sm_av(t):
                qb, pair, kp = units[t]
                n_kp = 2 * (qb + 1)
                o = kp - (n_kp - 2)
                qv0 = 256 if o == 1 else 0
                if kp == 0:
                    av_tiles[(qb, pair)] = [
                        av_ps.tile([128, 512], f32, tag="av", name=f"av{u}")
                        for u in range(2)]
                avs = av_tiles[(qb, pair)]
                stps = st_tiles.pop(t)
                for u in range(2):
                    pt = ptp.tile([128, 2, 512], bf16, tag="pt")
                    nc.scalar.activation(pt[:, :, qv0:512],
                                         stps[u][:, :, qv0:512], EXP,
                                         scale=float(HD) ** -0.5)
                    if o >= 0:
                        mk2 = masks[:, o, :].rearrange("p (i c) -> p i c",
                                                       i=2)
                        nc.vector.tensor_mul(pt[:, :, qv0:512],
                                             pt[:, :, qv0:512],
                                             mk2[:, :, qv0:512])
                    for i in range(2):
                        kt = 2 * kp + i
                        # valid q range of this k-subtile (av accumulate is
                        # sub-range safe; masked-out region contributes 0)
                        qa = qv0 if o < 0 else 128 * (2 * o + i)
                        nc.tensor.matmul(
                            avs[u][:, qa:512],
                            lhsT=vE[:, u * 16 + kt, pair, :],
                            rhs=pt[:, i, qa:512],
                            start=(kp == 0 and i == 0),
                            stop=(kp == n_kp - 1 and i == 1))

            def emit_norm(qb, pair):
                avs = av_tiles.pop((qb, pair))
                if qb < 3:
                    a2v = ab_in[qb].rearrange("(j f) c -> f j c", f=192)
                    a2v = a2v[64 * pair:64 * (pair + 1), :, :]
                elif pair < 2:
                    a2v = a3a_in.rearrange("(j f) c -> f j c", f=128)
                    a2v = a2v[64 * pair:64 * (pair + 1), :, :]
                else:
                    a2v = a3b_in.rearrange("(j f) c -> f j c", f=64)
                for u in range(2):
                    # NB: reciprocal_approx_fast is a custom-DVE op and can
                    # NOT read PSUM -- copy the denominator to SBUF first
                    lsb = sm.tile([64, 512], f32, tag="lsb", name=f"lsb{u}")
                    nc.vector.tensor_copy(lsb[:], avs[u][64:128, :])
                    rec = sm.tile([64, 512], f32, tag="rec", name=f"rec{u}")
                    nc.vector.reciprocal_approx_fast(rec[:], lsb[:])
                    ctxn = sm.tile([64, 512], bf16, tag="ctxn",
                                   name=f"ctxn{u}")
                    nc.vector.tensor_mul(ctxn[:], avs[u][0:64, :], rec[:])
                    nc.sync.dma_start(
                        a2v[:, 4 * u:4 * u + 4, :],
                        ctxn.rearrange("p (q c) -> p q c", q=4))

            def emit_coll(ins, outs):
                nc.gpsimd.collective_compute(
                    "AllToAll", mybir.AluOpType.bypass,
                    ins=[ins[:]], outs=[outs[:]], replica_groups=RG)

            def fetch_ag(ag, qb):
                nc.sync.dma_start(
                    ag[:], ab_out[qb].rearrange("(o p) r -> p o r", p=128))

            def emit_outproj_blk(qb, blk, ag, split_dma=True):
                osb = outp.tile([128, D], f32, tag="osb")
                for nh in range(2):
                    po = mix_ps.tile([128, 384], f32, tag="mix", name="po")
                    n0 = nh * 384
                    for j in range(KSUB):
                        nc.tensor.matmul(po[:],
                                         lhsT=ag[:, 6 * blk + j, :],
                                         rhs=wp[:, j, n0:n0 + 384],
                                         start=(j == 0),
                                         stop=(j == KSUB - 1))
                    nc.vector.tensor_add(osb[:, n0:n0 + 384], po[:],
                                         biasB[:, n0:n0 + 384])
                    if split_dma:
                        nc.sync.dma_start(out_d[qb, blk, :, n0:n0 + 384],
                                          osb[:, n0:n0 + 384])
                if not split_dma:
                    nc.sync.dma_start(out_d[qb, blk], osb[:])

            def emit_op3_part1(blk, agA, osb3):
                for nh in range(2):
                    po = mix_ps.tile([128, 384], f32, tag="mix", name="po")
                    n0 = nh * 384
                    for t in range(4):
                        nc.tensor.matmul(po[:],
                                         lhsT=agA[:, 4 * blk + t, :],
                                         rhs=wpP[:, t, n0:n0 + 384],
                                         start=(t == 0), stop=(t == 3))
                    nc.vector.tensor_add(osb3[blk][:, n0:n0 + 384], po[:],
                                         biasB[:, n0:n0 + 384])

            # ---- software-pipelined emission ---------------------------
            # (no warmup collective: AB(0) absorbs the ncfw first-call cost;
            # its results are not needed until q-block 3's filler window)
            # prologue: everything attention qb0 needs
            for b in range(2):
                for ct in range(3):
                    emit_qk_ct(b, 0, ct, first=(b == 0 and ct == 0))
            for rt in range(4):
                emit_v_tile(0, rt)
                emit_v_tile(1, rt)

            # filler queue: (deadline=emission step, emit_fn); before each
            # unit's S_T all fillers due by then are drained (hard ordering
            # requirement: a filler must be emitted before the attention
            # that consumes its output), plus one opportunistically per
            # step to spread PE filler.
            import heapq
            fqh = []
            fqseq = [0]

            def fq_push(dl, fn):
                heapq.heappush(fqh, (dl, fqseq[0], fn))
                fqseq[0] += 1

            for rc in range(1, 4):
                for b in range(2):
                    for ct in range(3):
                        # qT/kT rows rc needed from the first unit of qb=rc;
                        # its S_T is emitted one step early
                        fq_push(uidx[(rc, 0, 0)] - 1,
                                lambda b=b, rc=rc, ct=ct:
                                emit_qk_ct(b, rc, ct))
                for rt in range(4 * rc, 4 * rc + 4):
                    for b in range(2):
                        # vE row-tile rt consumed at AV of kp=rt//2 of the
                        # first pair of qb=rc
                        fq_push(uidx[(rc, 0, max(0, rt // 2 - 1))],
                                lambda b=b, rt=rt: emit_v_tile(b, rt))

            def drain(n, due=None):
                k = 0
                while fqh and (k < n or (due is not None and fqh[0][0] <= due)):
                    heapq.heappop(fqh)[2]()
                    k += 1

            # out-projection of q-block qb: ag fetch ~half a q-block after
            # AB(qb) completes (its DMA is queue-gated on AB(qb) anyway);
            # the matmul blocks run as PE filler another half-block later
            FETCH_AT = {0: (2, 1, 1), 1: (2, 2, 2), 2: (3, 1, 0)}
            OP_AT = {0: [(3, 0, 0), (3, 0, 2)],
                     1: [(3, 0, 4), (3, 0, 6)],
                     2: [(3, 1, 3), (3, 1, 6)]}
            ags = {}

            emit_st(0)
            for t in range(NU):
                if t + 1 < NU:
                    drain(1, due=t + 1)
                    emit_st(t + 1)
                emit_sm_av(t)
                qb, pair, kp = units[t]
                if kp != 2 * (qb + 1) - 1:
                    continue
                # ---- last k-pair of (qb, pair) ----
                emit_norm(qb, pair)
                if qb == 3 and pair == 1:
                    # ship pairs 0+1 now; they fly during pair 2's attention
                    emit_coll(a3a_in, a3a_out)
                    agA = agp.tile([128, 8, 128], bf16, tag="agA")
                    osb3 = [outp.tile([128, D], f32, tag="osb",
                                      name=f"osb3_{blk}") for blk in range(2)]
                    fq_push(uidx[(3, 2, 4)], lambda: nc.sync.dma_start(
                        agA[:], a3a_out.rearrange("(o p) r -> p o r", p=128)))
                    for blk in range(2):
                        fq_push(uidx[(3, 2, 6 + blk)],
                                lambda blk=blk: emit_op3_part1(blk, agA, osb3))
                if pair == 2 and qb == 3:
                    drain(99, due=NU)   # everything out before the last coll
                    emit_coll(a3b_in, a3b_out)
                if pair == 2 and qb < 3:
                    emit_coll(ab_in[qb], ab_out[qb])
                    if True:
                        ag = agp.tile([128, 2 * KSUB, 128], bf16, tag="ag",
                                      name=f"ag{qb}")
                        ags[qb] = ag
                        fq_push(uidx[FETCH_AT[qb]],
                                lambda ag=ag, qb=qb: fetch_ag(ag, qb))
                        for blk, su in enumerate(OP_AT[qb]):
                            fq_push(uidx[su],
                                    lambda qb=qb, blk=blk, ag=ag:
                                    emit_outproj_blk(qb, blk, ag))
            # ---- epilogue: q-block 3 pair-2 slice + stores --------------
            agB = agp.tile([128, 4, 128], bf16, tag="agB")
            nc.sync.dma_start(agB[:],
                              a3b_out.rearrange("(o p) r -> p o r", p=128))
            for blk in range(2):
                for nh in range(2):
                    po = mix_ps.tile([128, 384], f32, tag="mix", name="po")
                    n0 = nh * 384
                    nc.tensor.matmul(po[:], lhsT=agB[:, 2 * blk, :],
                                     rhs=wpP[:, 4, n0:n0 + 384],
                                     start=True, stop=False)
                    nc.tensor.matmul(po[:], lhsT=agB[:, 2 * blk + 1, :],
                                     rhs=wpP[:, 5, n0:n0 + 384],
                                     start=False, stop=True)
                    nc.vector.tensor_add(osb3[blk][:, n0:n0 + 384], po[:],
                                         osb3[blk][:, n0:n0 + 384])
                    nc.sync.dma_start(out_d[3, blk, :, n0:n0 + 384],
                                      osb3[blk][:, n0:n0 + 384])

    nc.compile()
    return nc


def _get_nc():
    if "nc" not in _CACHE:
        _CACHE["nc"] = _build_nc()
    return _CACHE["nc"]


def _masks_np():
    k = np.arange(128)[:, None]
    q = np.arange(512)[None, :]
    tiles = [(q >= k + 128 * t) for t in range(4)]
    m = np.stack([np.concatenate([tiles[2 * o], tiles[2 * o + 1]], axis=1)
                  for o in range(2)])
    return m.astype(BF16)


def _prep_in_maps(x, Wq, Wk, Wv, Wp, bp):
    x = np.asarray(x, dtype=np.float32)
    mk = _masks_np()
    # Natural Wp row order: the merged per-q-block AllToAll delivers each
    # sender's 192 rows contiguously, and 4 senders x 192 = 6 x 128-row
    # matmul chunks in plain head order.
    wpa = np.asarray(Wp)
    wp_full = wpa.astype(BF16)
    wpP_full = np.concatenate(
        [wpa[192 * h:192 * h + 128] for h in range(4)]
        + [wpa[192 * h + 128:192 * h + 192] for h in range(4)],
        axis=0).astype(BF16)
    bp_row = np.asarray(bp, dtype=np.float32).reshape(1, D).astype(BF16)
    xT_bg = []
    for bg in range(2):
        xl = x[2 * bg:2 * bg + 2].reshape(R, D)
        xT_bg.append(np.ascontiguousarray(xl.T).astype(BF16))
    wqk_hg, wv_hg = [], []
    for hg in range(4):
        hs = slice(192 * hg, 192 * (hg + 1))
        wqk_hg.append(np.concatenate(
            [np.asarray(Wq)[:, hs], np.asarray(Wk)[:, hs]], axis=1).astype(BF16))
        wv_hg.append(np.asarray(Wv)[:, hs].astype(BF16))
    in_maps = []
    for c in range(N_CORES):
        bg, hg = c // 4, c % 4
        in_maps.append({
            "xT": xT_bg[bg],
            "wqk": wqk_hg[hg],
            "wv": wv_hg[hg],
            "wp": wp_full,
            "wpP": wpP_full,
            "bp": bp_row,
            "mk": mk,
        })
    return in_maps


def kernel(x, Wq, Wk, Wv, Wp, bp):
    from concourse import bass_utils

    nc = _get_nc()
    in_maps = _prep_in_maps(x, Wq, Wk, Wv, Wp, bp)
    res = bass_utils.run_bass_kernel_spmd(nc, in_maps,
                                          core_ids=list(range(N_CORES)))
    out = np.empty((B, S, D), np.float32)
    for c in range(N_CORES):
        sh = res.results[c]["out"]  # [4 chunks, 2 blocks, 128, D]
        for qb in range(4):
            for blk in range(2):
                batch = 2 * blk + c // 4
                s0 = 512 * qb + 128 * (c % 4)
                out[batch, s0:s0 + 128] = sh[qb, blk]
    return out
